# revision 34
# baseline (speedup 1.0000x reference)
"""Causal self-attention (B=4, T=2048, C=1024, H=16) on 8 Trainium2 cores.

Sharding: batch x head-half. Core c handles batch b=c//2 and heads
[8*(c%2), 8*(c%2)+8). Each core computes qkv for its head slice, causal
flash-style attention fully in SBUF, and a partial c_proj; a pairwise
ReduceScatter (cores 2b, 2b+1) sums the two head-halves and leaves each
core with 1024 rows of the final output.

Schedule: one shared 2-buffer psum ring carries every non-PV matmul
output (v, qk, S, proj).  The front wavefront advances v, group-0 qk
and group-0 attention together per 512-token chunk as x streams in, so
ACT gets exp work early; later groups' qk matmuls are queued as filler
chunks popped between attention batches to keep the PE busy while ACT
is the attention bottleneck.  Group 3's attention interleaves with the
per-q-block proj + ReduceScatter so the collectives spread out.

Layouts (per core):
  xT   [1024 C, 2048 tok]   host-pre-transposed bf16
  qT,kT [512 cols, 2048]    bf16, computed as W^T-stationary matmuls
  v_aug [tok, 8 heads x 65] bf16 (65th col = 1.0 -> softmax denominator)
  S^T  [128 ktok, 2x512 q]  psum pairs; exp on ACT -> attT bf16 (causal
                            mask via one strided mul on the 2 diagonals)
  PV   attT^T @ v_aug -> [128 q, 65] psum accumulated over ktiles; col 64
       is the softmax denominator; normalize with DVE reciprocal
  y -> yT via PE transpose (bf16); proj = yT-stationary matmuls + bias
"""
import os
import time
from contextlib import ExitStack

import numpy as np
import ml_dtypes

import concourse.bass as bass
import concourse.mybir as mybir
import concourse.tile as tile
from concourse.ap import AP
from concourse.masks import make_identity


def strided(ap, offset, dims):
    """AP with explicit free dims [[stride, count], ...] at elem offset."""
    pdim = list(map(list, ap.ap))[0]
    return AP(ap.tensor, ap.offset + offset, [pdim] + dims)

B, T, C = 4, 2048, 1024
H, HD = 16, 64
NCORES = 8
P = 128
KC = C // P  # 8 contraction chunks
HPC = H // 2  # heads per core
HCOLS = HPC * HD  # 512 qkv columns per core
TOKTILES = T // P  # 16
F32 = mybir.dt.float32
F32R = mybir.dt.float32r
BF16 = mybir.dt.bfloat16
# x and weights travel as bf16: halves the HBM load traffic and SBUF
# footprint; matmul rate is 1 cycle/row either way
IO_BF16 = True
IO_DT = BF16 if IO_BF16 else F32R


def legalize_waits(nc):
    """This walrus build rejects >1 sem wait per instruction (>2 for
    EventSemaphore): split extras onto preceding same-engine NOPs."""
    for f in nc.m.functions:
        for bb in f.blocks:
            new_insts = []
            for inst in bb.instructions:
                si = inst.sync_info
                cap = 2 if isinstance(inst, mybir.InstEventSemaphore) else 1
                if si is not None and si.on_wait and len(si.on_wait) > cap:
                    waits = list(si.on_wait)
                    extra, keep = waits[:-cap], waits[-cap:]
                    for k, w in enumerate(extra):
                        new_insts.append(
                            mybir.InstNoOp(
                                name=f"{inst.name}-splitw{k}",
                                engine=inst.engine,
                                sync_info=mybir.SyncInfo(on_wait=[w], on_update=[]),
                            )
                        )
                    si.on_wait = keep
                    inst.sync_info = si
                new_insts.append(inst)
            bb.instructions = new_insts


def build_nc(
    reps: int = 1,
    rs_bf16: bool = True,
    no_rs: bool = False,
    upto: int = 3,
):
    nc = bass.Bass()
    xt_in = nc.declare_dram_parameter("xt", [C, T], IO_DT, isOutput=False)
    w3_in = nc.declare_dram_parameter("w3", [C, 3 * HCOLS], IO_DT, isOutput=False)
    wp_in = nc.declare_dram_parameter("wp", [HCOLS, C], IO_DT, isOutput=False)
    bq_in = nc.declare_dram_parameter("bq", [4, P, 1], F32, isOutput=False)
    bk_in = nc.declare_dram_parameter("bk", [4, P, 1], F32, isOutput=False)
    bvb_in = nc.declare_dram_parameter("bvb", [P, HCOLS], F32, isOutput=False)
    bpb_in = nc.declare_dram_parameter("bpb", [P, C], F32, isOutput=False)
    masks_in = nc.declare_dram_parameter("masks", [1, P, P], BF16, isOutput=False)
    out_p = nc.declare_dram_parameter(
        "out_part", [T // 2, C], BF16 if rs_bf16 else F32, isOutput=True
    )

    with tile.TileContext(nc) as tc, ExitStack() as top:
        dram = top.enter_context(tc.tile_pool(name="dram", bufs=1, space="DRAM"))
        rs_dt = BF16 if rs_bf16 else F32
        partial = [dram.tile([512, C], rs_dt, tag=f"partial{g}", name=f"partial{g}") for g in range(4)]
        rs_out = [dram.tile([256, C], rs_dt, tag=f"rs{g}", name=f"rs{g}") for g in range(4)]

        const = top.enter_context(tc.tile_pool(name="const", bufs=1))
        # triangle mask duplicated side by side for strided 2-block muls
        trimask = const.tile([P, 2, P], BF16, name="trimask")
        nc.sync.dma_start(trimask[:, 0, :], masks_in[0])
        nc.sync.dma_start(trimask[:, 1, :], masks_in[0])
        bq_t = [const.tile([P, 1], F32, tag=f"bq{m}", name=f"bq{m}") for m in range(4)]
        bk_t = [const.tile([P, 1], F32, tag=f"bk{m}", name=f"bk{m}") for m in range(4)]
        for m in range(4):
            nc.sync.dma_start(bq_t[m][:], bq_in[m])
            nc.sync.dma_start(bk_t[m][:], bk_in[m])
        bvb = const.tile([P, HCOLS], F32)
        nc.sync.dma_start(bvb[:], bvb_in[:])
        bpb = const.tile([P, C], F32)
        nc.sync.dma_start(bpb[:], bpb_in[:])
        ident = const.tile([P, P], IO_DT)
        make_identity(nc, ident[:])

        def body():
            with ExitStack() as ctx:
                # ---- persistent SBUF for this iteration ----
                qkv_pool = ctx.enter_context(tc.tile_pool(name="qkv", bufs=1))
                qT = [qkv_pool.tile([P, T], BF16, tag=f"qT{m}", name=f"qT{m}") for m in range(4)]
                kT = [qkv_pool.tile([P, T], BF16, tag=f"kT{m}", name=f"kT{m}") for m in range(4)]
                vaug = [
                    qkv_pool.tile([P, HPC, HD + 1], BF16, tag=f"v{t}", name=f"v{t}")
                    for t in range(TOKTILES)
                ]
                y_pool = ctx.enter_context(tc.tile_pool(name="y", bufs=1))
                y = [y_pool.tile([P, HCOLS], IO_DT, tag=f"y{t}", name=f"y{t}") for t in range(TOKTILES)]

                with ExitStack() as qctx:
                    # attention pools (small, live for the whole body)
                    att_sb = qctx.enter_context(tc.tile_pool(name="att_sb", bufs=4))
                    sm_pool = qctx.enter_context(tc.tile_pool(name="sm", bufs=4))

                    # qk matmul chunks ride the shared sp psum ring between
                    # attention batches: the queue paces one chunk per few
                    # batches so the PE always has work while ACT runs exp
                    fillq = []
                    bc = [0]

                    def pop_filler():
                        bc[0] += 1
                        if fillq and bc[0] % 3 == 0:
                            fillq.pop(0)()

                    def drain_fillers():
                        while fillq:
                            fillq.pop(0)()

                    def attention(m, h, qc):
                        hsl = slice((h % 2) * HD, (h % 2) * HD + HD)
                        nb = 2 * qc + 2  # batches of 2 key-tiles
                        pv = [
                            pv_ps.tile([P, HD + 1], F32, tag=f"pv{sb}", name=f"pv{sb}")
                            for sb in range(4)
                        ]
                        ats = [None] * nb

                        def emit_pv(b):
                            for j in (0, 1):
                                kt = 2 * b + j
                                for sb in range(4):
                                    # last contributing tile for sub sb is
                                    # kt == qc*4+sb (later tiles fully masked)
                                    if kt > qc * 4 + sb:
                                        continue
                                    nc.tensor.matmul(
                                        pv[sb][:],
                                        ats[b][:, j * 512 + sb * P : j * 512 + (sb + 1) * P],
                                        vaug[kt][:, h, :],
                                        start=(kt == 0),
                                        stop=(kt == qc * 4 + sb),
                                    )

                        # software pipeline: S one batch ahead of PV so the
                        # PE never stalls waiting for ACT's exp
                        for b in range(nb):
                            d_lo = max(0, 256 * b - 512 * qc)
                            sp = s_ps.tile([P, 1024], F32, tag="sp", name="sp")
                            for j in (0, 1):
                                kt = 2 * b + j
                                d = max(0, kt * P - qc * 512)
                                nc.tensor.matmul(
                                    sp[:, j * 512 + d : (j + 1) * 512],
                                    kT[m][hsl, kt * P : (kt + 1) * P],
                                    qT[m][hsl, qc * 512 + d : (qc + 1) * 512],
                                    start=True,
                                    stop=True,
                                )
                            at = att_sb.tile([P, 1024], BF16, tag="at", name="at")
                            if b < 2 * qc:
                                # both tiles full: one exp over both blocks
                                nc.scalar.activation(
                                    strided(at[:], 0, [[512, 2], [1, 512]]),
                                    strided(sp[:], 0, [[512, 2], [1, 512]]),
                                    mybir.ActivationFunctionType.Exp,
                                    scale=0.125,
                                )
                            else:
                                # diagonal tiles have different valid widths
                                for j in (0, 1):
                                    dj = max(0, (2 * b + j) * P - qc * 512)
                                    nc.scalar.activation(
                                        at[:, j * 512 + dj : (j + 1) * 512],
                                        sp[:, j * 512 + dj : (j + 1) * 512],
                                        mybir.ActivationFunctionType.Exp,
                                        scale=0.125,
                                    )
                            if b >= 2 * qc:
                                # both tiles are diagonal: triangle blocks at
                                # cols d_lo and d_lo+640 (one strided mul)
                                mav = strided(at[:], d_lo, [[640, 2], [1, P]])
                                nc.vector.tensor_mul(mav, mav, trimask[:])
                            ats[b] = at
                            if b >= 1:
                                emit_pv(b - 1)
                            pop_filler()
                        emit_pv(nb - 1)
                        for sb in range(4):
                            t = qc * 4 + sb
                            rec = sm_pool.tile([P, 1], F32, tag="rec", name="rec")
                            nc.vector.reciprocal(rec[:], pv[sb][:, HD : HD + 1])
                            nc.vector.tensor_scalar_mul(
                                y[t][:, h * HD : (h + 1) * HD],
                                pv[sb][:, 0:HD],
                                rec[:],
                            )

                    with ExitStack() as xctx:
                        xt_pool = xctx.enter_context(tc.tile_pool(name="xt", bufs=1))
                        xT = [xt_pool.tile([P, T], IO_DT, tag=f"xT{k}", name=f"xT{k}") for k in range(KC)]
                        stage = xctx.enter_context(tc.tile_pool(name="stage", bufs=2))

                        # DMA queue order: v weights, first x chunk, group-0
                        # qk weights, rest of x — so the first v matmuls and
                        # the first attention group start as early as possible
                        wrv = [
                            stage.tile([P, KC, 256], IO_DT, tag=f"wrv{cc}", name=f"wrv{cc}")
                            for cc in range(2)
                        ]
                        for cc in range(2):
                            nc.sync.dma_start(
                                wrv[cc][:],
                                w3_in[
                                    :, 2 * HCOLS + cc * 256 : 2 * HCOLS + (cc + 1) * 256
                                ].rearrange("(kc p) m -> p kc m", p=P),
                            )

                        def emit_x_dma(tck):
                            for k in range(KC):
                                nc.sync.dma_start(
                                    xT[k][:, tck * 512 : (tck + 1) * 512],
                                    xt_in[k * P : (k + 1) * P, tck * 512 : (tck + 1) * 512],
                                )

                        def emit_wr_dma(m, part):
                            wr = stage.tile([P, KC, P], IO_DT, tag="wr", bufs=3, name="wr")
                            col0 = part * HCOLS + m * P
                            nc.sync.dma_start(
                                wr[:],
                                w3_in[:, col0 : col0 + P].rearrange(
                                    "(kc p) m -> p kc m", p=P
                                ),
                            )
                            return wr

                        emit_x_dma(0)
                        wr0 = [emit_wr_dma(0, 0), emit_wr_dma(0, 1)]
                        for tck in range(1, 4):
                            emit_x_dma(tck)

                        # unified psum layout for the whole body: one shared
                        # sp ring 2x[128,1024] (4 banks) carries v, qk, S and
                        # proj tiles; pv takes the other 4 banks
                        s_ps = qctx.enter_context(
                            tc.tile_pool(name="s_ps", bufs=2, space="PSUM")
                        )
                        pv_ps = qctx.enter_context(
                            tc.tile_pool(name="pv_ps", bufs=1, space="PSUM")
                        )

                        def v_tile(t):
                            nc.vector.memset(vaug[t][:, :, HD : HD + 1], 1.0)
                            for cc in range(2):
                                pt = s_ps.tile([P, 256], F32, tag="sp", name="pt")
                                for k in range(KC):
                                    nc.tensor.matmul(
                                        pt[:],
                                        xT[k][:, t * P : (t + 1) * P],
                                        wrv[cc][:, k, :],
                                        start=(k == 0),
                                        stop=(k == KC - 1),
                                    )
                                nc.vector.tensor_add(
                                    vaug[t][:, cc * 4 : (cc + 1) * 4, 0:HD],
                                    pt[:].rearrange("p (h d) -> p h d", d=HD),
                                    bvb[:, cc * 256 : (cc + 1) * 256].rearrange(
                                        "p (h d) -> p h d", d=HD
                                    ),
                                )

                        def qk_chunk(m, part, tck, wr):
                            dst = (qT, kT)[part]
                            bias = (bq_t, bk_t)[part]
                            pt = s_ps.tile([P, 512], F32, tag="sp", name="pt")
                            for k in range(KC):
                                nc.tensor.matmul(
                                    pt[:],
                                    wr[:, k, :],
                                    xT[k][:, tck * 512 : (tck + 1) * 512],
                                    start=(k == 0),
                                    stop=(k == KC - 1),
                                )
                            nc.vector.tensor_scalar_add(
                                dst[m][:, tck * 512 : (tck + 1) * 512],
                                pt[:],
                                bias[m][:],
                            )

                        def queue_qk(m):
                            for part in (0, 1):
                                wr = emit_wr_dma(m, part)
                                fillq.extend(
                                    lambda mm=m, pp=part, tt=tck, ww=wr: qk_chunk(
                                        mm, pp, tt, ww
                                    )
                                    for tck in range(4)
                                )

                        # front wavefront: v, group-0 qk and group-0
                        # attention advance together per token chunk, so ACT
                        # gets exp work ~30us earlier than a phased schedule
                        for tck in range(4):
                            for t in range(4 * tck, 4 * tck + 4):
                                v_tile(t)
                            qk_chunk(0, 0, tck, wr0[0])
                            qk_chunk(0, 1, tck, wr0[1])
                            if tck == 2:
                                queue_qk(1)
                            if upto >= 2:
                                attention(0, 0, tck)
                                attention(0, 1, tck)

                        for m in (1, 2):
                            drain_fillers()
                            queue_qk(m + 1)
                            if upto >= 2:
                                for qc in range(4):
                                    for h in (2 * m, 2 * m + 1):
                                        attention(m, h, qc)
                        drain_fillers()

                    if upto < 2:
                        return
                    # xT and staging freed; bring in proj buffers
                    yT = qctx.enter_context(tc.tile_pool(name="yt", bufs=1)).tile(
                        [P, 4, T], IO_DT, name="yT"
                    )
                    wpr = qctx.enter_context(tc.tile_pool(name="wpr", bufs=1)).tile(
                        [P, 4, C], IO_DT, name="wpr"
                    )
                    nc.sync.dma_start(
                        wpr[:], wp_in[:].rearrange("(kc p) m -> p kc m", p=P)
                    )
                    ob_pool = qctx.enter_context(tc.tile_pool(name="ob", bufs=3))

                    def proj_group(qc):
                        # transpose y tiles of this group, then proj + bias,
                        # then one pairwise reduce-scatter for the group
                        for sb in range(4):
                            t = qc * 4 + sb
                            tp = s_ps.tile([P, 512], IO_DT, tag="sp", name="tp")
                            for m in range(4):
                                nc.tensor.transpose(
                                    tp[:, m * P : (m + 1) * P],
                                    y[t][:, m * P : (m + 1) * P],
                                    ident[:],
                                )
                            # scatter the four column-chunks into yT[:, m, t]
                            nc.vector.tensor_copy(
                                strided(yT[:], t * P, [[T, 4], [1, P]]),
                                tp[:].rearrange("p (m c) -> p m c", c=P),
                            )
                        for sb in range(4):
                            t = qc * 4 + sb
                            pt = s_ps.tile([P, 1024], F32, tag="sp", name="pt")
                            for ncol in range(2):
                                for k in range(4):
                                    nc.tensor.matmul(
                                        pt[:, ncol * 512 : (ncol + 1) * 512],
                                        yT[:, k, t * P : (t + 1) * P],
                                        wpr[:, k, ncol * 512 : (ncol + 1) * 512],
                                        start=(k == 0),
                                        stop=(k == 3),
                                    )
                            ob = ob_pool.tile([P, C], rs_dt, tag="ob", name="ob")
                            nc.vector.tensor_add(ob[:], pt[:], bpb[:])
                            nc.sync.dma_start(
                                partial[qc][sb * P : (sb + 1) * P, :], ob[:]
                            )
                        if not no_rs:
                            nc.gpsimd.collective_compute(
                                "ReduceScatter",
                                mybir.AluOpType.add,
                                replica_groups=[[0, 1], [2, 3], [4, 5], [6, 7]],
                                ins=[partial[qc].opt()],
                                outs=[rs_out[qc].opt()],
                            )
                        nc.sync.dma_start(
                            out_p[qc * 256 : (qc + 1) * 256, :], rs_out[qc][:]
                        )

                    # group 3 attention with inline proj + reduce-scatter
                    for qc in range(4):
                        for h in (6, 7):
                            attention(3, h, qc)
                        if upto >= 3:
                            proj_group(qc)

        for _ in range(reps):
            body()

    legalize_waits(nc)
    return nc


def prep_inputs(x, W_qkv, b_qkv, W_proj, b_proj):
    x = np.asarray(x, dtype=np.float32)
    W_qkv = np.asarray(W_qkv, dtype=np.float32)
    b_qkv = np.asarray(b_qkv, dtype=np.float32)
    W_proj = np.asarray(W_proj, dtype=np.float32)
    b_proj = np.asarray(b_proj, dtype=np.float32)

    io_np = ml_dtypes.bfloat16 if IO_BF16 else np.float32
    xTs = [np.ascontiguousarray(x[b].T.astype(io_np)) for b in range(B)]
    halves = []
    for half in range(2):
        c0 = half * HCOLS
        w3 = np.ascontiguousarray(
            np.concatenate(
                [
                    W_qkv[:, c0 : c0 + HCOLS],
                    W_qkv[:, C + c0 : C + c0 + HCOLS],
                    W_qkv[:, 2 * C + c0 : 2 * C + c0 + HCOLS],
                ],
                axis=1,
            ).astype(io_np)
        )
        wp = np.ascontiguousarray(W_proj[c0 : c0 + HCOLS, :].astype(io_np))
        bq = np.ascontiguousarray(b_qkv[c0 : c0 + HCOLS].reshape(4, P, 1))
        bk = np.ascontiguousarray(b_qkv[C + c0 : C + c0 + HCOLS].reshape(4, P, 1))
        bvb = np.tile(b_qkv[2 * C + c0 : 2 * C + c0 + HCOLS], (P, 1))
        halves.append((w3, wp, bq, bk, np.ascontiguousarray(bvb)))
    # both cores of a pair add the proj bias before the ReduceScatter
    # sums them, so each adds half
    bpb = np.ascontiguousarray(np.tile(b_proj / 2.0, (P, 1)))

    kk = np.arange(P)[:, None]
    qq = np.arange(P)[None, :]
    masks = (kk <= qq).astype(ml_dtypes.bfloat16)[None]

    in_maps = []
    for c in range(NCORES):
        b, half = c // 2, c % 2
        w3, wp, bq, bk, bvb = halves[half]
        in_maps.append(
            {
                "xt": xTs[b],
                "w3": w3,
                "wp": wp,
                "bq": bq,
                "bk": bk,
                "bvb": bvb,
                "bpb": bpb,
                "masks": masks,
            }
        )
    return in_maps


class _Runner:
    """Build-once SPMD executor via PJRT (mirrors bass2jax.run_bass_via_pjrt)."""

    def __init__(self, nc, n_cores=NCORES):
        import jax
        from jax.sharding import Mesh, PartitionSpec, NamedSharding
        from jax.experimental.shard_map import shard_map
        from concourse.bass2jax import (
            _bass_exec_p,
            install_neuronx_cc_hook,
            partition_id_tensor,
        )

        self.jax = jax
        install_neuronx_cc_hook()
        partition_name = (
            nc.partition_id_tensor.name if nc.partition_id_tensor else None
        )
        in_names, out_names, out_avals, zero_outs = [], [], [], []
        for alloc in nc.m.functions[0].allocations:
            if not isinstance(alloc, mybir.MemoryLocationSet):
                continue
            name = alloc.memorylocations[0].name
            if alloc.kind == "ExternalInput":
                if name != partition_name:
                    in_names.append(name)
            elif alloc.kind == "ExternalOutput":
                shape = tuple(alloc.tensor_shape)
                dtype = mybir.dt.np(alloc.dtype)
                out_names.append(name)
                out_avals.append(jax.core.ShapedArray(shape, dtype))
                zero_outs.append(np.zeros(shape, dtype))
        self.in_names, self.out_names = in_names, out_names
        self.out_avals, self.zero_outs = out_avals, zero_outs
        self.n_cores = n_cores
        n_params = len(in_names)
        self.n_params = n_params
        all_in = list(in_names) + list(out_names)
        if partition_name is not None:
            all_in.append(partition_name)
        donate = tuple(range(n_params, n_params + len(out_names)))

        def _body(*args):
            operands = list(args)
            if partition_name is not None:
                operands.append(partition_id_tensor())
            outs = _bass_exec_p.bind(
                *operands,
                out_avals=tuple(out_avals),
                in_names=tuple(all_in),
                out_names=tuple(out_names),
                lowering_input_output_aliases=(),
                sim_require_finite=True,
                sim_require_nnan=True,
                nc=nc,
            )
            return tuple(outs)

        devices = jax.devices()[:n_cores]
        self.mesh = Mesh(np.asarray(devices), ("core",))
        in_specs = (PartitionSpec("core"),) * (n_params + len(out_names))
        out_specs = (PartitionSpec("core"),) * len(out_names)
        self.sharding = NamedSharding(self.mesh, PartitionSpec("core"))
        self.jitted = jax.jit(
            shard_map(
                _body,
                mesh=self.mesh,
                in_specs=in_specs,
                out_specs=out_specs,
                check_rep=False,
            ),
            donate_argnums=donate,
            keep_unused=True,
        )

    def put_inputs(self, in_maps):
        per_core = [[np.asarray(m[n]) for n in self.in_names] for m in in_maps]
        concat = [
            np.concatenate([per_core[c][i] for c in range(self.n_cores)], axis=0)
            for i in range(self.n_params)
        ]
        return [self.jax.device_put(a, self.sharding) for a in concat]

    def _zeros(self):
        return [
            self.jax.device_put(
                np.zeros((self.n_cores * z.shape[0], *z.shape[1:]), z.dtype),
                self.sharding,
            )
            for z in self.zero_outs
        ]

    def run(self, dev_inputs, n_timed=0):
        out = self.jitted(*dev_inputs, *self._zeros())
        self.jax.block_until_ready(out)
        times = []
        for _ in range(n_timed):
            z = self._zeros()
            self.jax.block_until_ready(z)
            t0 = time.perf_counter()
            out2 = self.jitted(*dev_inputs, *z)
            self.jax.block_until_ready(out2)
            times.append(time.perf_counter() - t0)
            out = out2
        np_outs = [np.asarray(a) for a in out]
        results = [
            {
                n: np_outs[i].reshape(self.n_cores, *self.out_avals[i].shape)[c]
                for i, n in enumerate(self.out_names)
            }
            for c in range(self.n_cores)
        ]
        return results, times


_RUNNERS = {}


def get_runner(reps: int = 1, **kw) -> _Runner:
    key = (reps, tuple(sorted(kw.items())))
    if key not in _RUNNERS:
        _RUNNERS[key] = _Runner(build_nc(reps, **kw))
    return _RUNNERS[key]


def kernel(x, W_qkv, b_qkv, W_proj, b_proj):
    in_maps = prep_inputs(x, W_qkv, b_qkv, W_proj, b_proj)
    runner = get_runner(1)
    results, _ = runner.run(runner.put_inputs(in_maps))
    out = np.empty((B, T, C), dtype=np.float32)
    for c in range(NCORES):
        b, rank = c // 2, c % 2
        part = results[c]["out_part"]
        for g in range(4):
            r0 = g * 512 + rank * 256
            out[b, r0 : r0 + 256, :] = np.asarray(
                part[g * 256 : (g + 1) * 256, :], dtype=np.float32
            )
    return out



# revision 37
# speedup vs baseline: 1.5444x; 1.5444x over previous
"""Causal self-attention (B=4, T=2048, C=1024, H=16) on 8 Trainium2 cores.

Sharding: batch x head-half. Core c handles batch b=c//2 and heads
[8*(c%2), 8*(c%2)+8). Each core computes qkv for its head slice, causal
flash-style attention fully in SBUF, and a partial c_proj; a pairwise
ReduceScatter (cores 2b, 2b+1) sums the two head-halves and leaves each
core with 1024 rows of the final output.

Schedule: one shared 2-buffer psum ring carries every non-PV matmul
output (v, qk, S, proj).  The front wavefront advances v, group-0 qk
and group-0 attention together per 512-token chunk as x streams in, so
ACT gets exp work early; later groups' qk matmuls are queued as filler
chunks popped between attention batches to keep the PE busy while ACT
is the attention bottleneck.  Group 3's attention interleaves with the
per-q-block proj + ReduceScatter so the collectives spread out.

Layouts (per core):
  xT   [1024 C, 2048 tok]   host-pre-transposed bf16
  qT,kT [512 cols, 2048]    bf16, computed as W^T-stationary matmuls
  v_aug [tok, 8 heads x 65] bf16 (65th col = 1.0 -> softmax denominator)
  S^T  [128 ktok, 2x512 q]  psum pairs; exp on ACT -> attT bf16 (causal
                            mask via one strided mul on the 2 diagonals)
  PV   attT^T @ v_aug -> [128 q, 65] psum accumulated over ktiles; col 64
       is the softmax denominator; normalize with DVE reciprocal
  y -> yT via PE transpose (bf16); proj = yT-stationary matmuls + bias
"""
import os
import time
from contextlib import ExitStack

import numpy as np
import ml_dtypes

import concourse.bass as bass
import concourse.mybir as mybir
import concourse.tile as tile
from concourse.ap import AP
from concourse.masks import make_identity


def strided(ap, offset, dims):
    """AP with explicit free dims [[stride, count], ...] at elem offset."""
    pdim = list(map(list, ap.ap))[0]
    return AP(ap.tensor, ap.offset + offset, [pdim] + dims)

B, T, C = 4, 2048, 1024
H, HD = 16, 64
NCORES = 8
P = 128
KC = C // P  # 8 contraction chunks
HPC = H // 2  # heads per core
HCOLS = HPC * HD  # 512 qkv columns per core
TOKTILES = T // P  # 16
F32 = mybir.dt.float32
F32R = mybir.dt.float32r
BF16 = mybir.dt.bfloat16
# x and weights travel as bf16: halves the HBM load traffic and SBUF
# footprint; matmul rate is 1 cycle/row either way
IO_BF16 = True
IO_DT = BF16 if IO_BF16 else F32R


def legalize_waits(nc):
    """This walrus build rejects >1 sem wait per instruction (>2 for
    EventSemaphore): split extras onto preceding same-engine NOPs."""
    for f in nc.m.functions:
        for bb in f.blocks:
            new_insts = []
            for inst in bb.instructions:
                si = inst.sync_info
                cap = 2 if isinstance(inst, mybir.InstEventSemaphore) else 1
                if si is not None and si.on_wait and len(si.on_wait) > cap:
                    waits = list(si.on_wait)
                    extra, keep = waits[:-cap], waits[-cap:]
                    for k, w in enumerate(extra):
                        new_insts.append(
                            mybir.InstNoOp(
                                name=f"{inst.name}-splitw{k}",
                                engine=inst.engine,
                                sync_info=mybir.SyncInfo(on_wait=[w], on_update=[]),
                            )
                        )
                    si.on_wait = keep
                    inst.sync_info = si
                new_insts.append(inst)
            bb.instructions = new_insts


def build_nc(
    reps: int = 1,
    rs_bf16: bool = True,
    no_rs: bool = False,
    upto: int = 3,
):
    nc = bass.Bass()
    xt_in = nc.declare_dram_parameter("xt", [C, T], IO_DT, isOutput=False)
    w3_in = nc.declare_dram_parameter("w3", [C, 3 * HCOLS], IO_DT, isOutput=False)
    wp_in = nc.declare_dram_parameter("wp", [HCOLS, C], IO_DT, isOutput=False)
    bq_in = nc.declare_dram_parameter("bq", [4, P, 1], F32, isOutput=False)
    bk_in = nc.declare_dram_parameter("bk", [4, P, 1], F32, isOutput=False)
    bvb_in = nc.declare_dram_parameter("bvb", [P, HCOLS], F32, isOutput=False)
    bpb_in = nc.declare_dram_parameter("bpb", [P, C], F32, isOutput=False)
    masks_in = nc.declare_dram_parameter("masks", [1, P, P], BF16, isOutput=False)
    out_p = nc.declare_dram_parameter(
        "out_part", [T // 2, C], BF16 if rs_bf16 else F32, isOutput=True
    )

    with tile.TileContext(nc) as tc, ExitStack() as top:
        dram = top.enter_context(tc.tile_pool(name="dram", bufs=1, space="DRAM"))
        rs_dt = BF16 if rs_bf16 else F32
        partial = [dram.tile([512, C], rs_dt, tag=f"partial{g}", name=f"partial{g}") for g in range(4)]
        rs_out = [dram.tile([256, C], rs_dt, tag=f"rs{g}", name=f"rs{g}") for g in range(4)]

        const = top.enter_context(tc.tile_pool(name="const", bufs=1))
        # triangle mask duplicated side by side for strided 2-block muls
        trimask = const.tile([P, 2, P], BF16, name="trimask")
        nc.sync.dma_start(trimask[:, 0, :], masks_in[0])
        nc.sync.dma_start(trimask[:, 1, :], masks_in[0])
        bq_t = [const.tile([P, 1], F32, tag=f"bq{m}", name=f"bq{m}") for m in range(4)]
        bk_t = [const.tile([P, 1], F32, tag=f"bk{m}", name=f"bk{m}") for m in range(4)]
        for m in range(4):
            nc.sync.dma_start(bq_t[m][:], bq_in[m])
            nc.sync.dma_start(bk_t[m][:], bk_in[m])
        bvb = const.tile([P, HCOLS], F32)
        nc.sync.dma_start(bvb[:], bvb_in[:])
        bpb = const.tile([P, C], F32)
        nc.sync.dma_start(bpb[:], bpb_in[:])
        ident = const.tile([P, P], IO_DT)
        make_identity(nc, ident[:])

        def body():
            with ExitStack() as ctx:
                # ---- persistent SBUF for this iteration ----
                qkv_pool = ctx.enter_context(tc.tile_pool(name="qkv", bufs=1))
                qT = [qkv_pool.tile([P, T], BF16, tag=f"qT{m}", name=f"qT{m}") for m in range(4)]
                kT = [qkv_pool.tile([P, T], BF16, tag=f"kT{m}", name=f"kT{m}") for m in range(4)]
                vaug = [
                    qkv_pool.tile([P, HPC, HD + 1], BF16, tag=f"v{t}", name=f"v{t}")
                    for t in range(TOKTILES)
                ]
                y_pool = ctx.enter_context(tc.tile_pool(name="y", bufs=1))
                y = [y_pool.tile([P, HCOLS], IO_DT, tag=f"y{t}", name=f"y{t}") for t in range(TOKTILES)]

                with ExitStack() as qctx:
                    # attention pools (small, live for the whole body)
                    att_sb = qctx.enter_context(tc.tile_pool(name="att_sb", bufs=6))
                    sm_pool = qctx.enter_context(tc.tile_pool(name="sm", bufs=8))

                    # qk matmul chunks ride the shared sp psum ring between
                    # attention batches: the queue paces one chunk per few
                    # batches so the PE always has work while ACT runs exp
                    fillq = []
                    bc = [0]

                    def pop_filler():
                        bc[0] += 1
                        if fillq and bc[0] % 3 == 0:
                            fillq.pop(0)()

                    def drain_fillers():
                        while fillq:
                            fillq.pop(0)()

                    def attention(m, h, qc):
                        hsl = slice((h % 2) * HD, (h % 2) * HD + HD)
                        nb = 2 * qc + 2  # batches of 2 key-tiles
                        pv = [
                            pv_ps.tile([P, HD + 1], F32, tag=f"pv{sb}", name=f"pv{sb}")
                            for sb in range(4)
                        ]
                        ats = [None] * nb

                        def emit_pv(b):
                            for j in (0, 1):
                                kt = 2 * b + j
                                for sb in range(4):
                                    # last contributing tile for sub sb is
                                    # kt == qc*4+sb (later tiles fully masked)
                                    if kt > qc * 4 + sb:
                                        continue
                                    nc.tensor.matmul(
                                        pv[sb][:],
                                        ats[b][:, j * 512 + sb * P : j * 512 + (sb + 1) * P],
                                        vaug[kt][:, h, :],
                                        start=(kt == 0),
                                        stop=(kt == qc * 4 + sb),
                                    )

                        # software pipeline: S one batch ahead of PV so the
                        # PE never stalls waiting for ACT's exp
                        for b in range(nb):
                            d_lo = max(0, 256 * b - 512 * qc)
                            sp = s_ps.tile([P, 1024], F32, tag="sp", name="sp")
                            for j in (0, 1):
                                kt = 2 * b + j
                                d = max(0, kt * P - qc * 512)
                                nc.tensor.matmul(
                                    sp[:, j * 512 + d : (j + 1) * 512],
                                    kT[m][hsl, kt * P : (kt + 1) * P],
                                    qT[m][hsl, qc * 512 + d : (qc + 1) * 512],
                                    start=True,
                                    stop=True,
                                )
                            at = att_sb.tile([P, 1024], BF16, tag="at", name="at")
                            if b < 2 * qc:
                                # both tiles full: one exp over both blocks
                                nc.scalar.activation(
                                    strided(at[:], 0, [[512, 2], [1, 512]]),
                                    strided(sp[:], 0, [[512, 2], [1, 512]]),
                                    mybir.ActivationFunctionType.Exp,
                                    scale=0.125,
                                )
                            else:
                                # diagonal tiles have different valid widths
                                for j in (0, 1):
                                    dj = max(0, (2 * b + j) * P - qc * 512)
                                    nc.scalar.activation(
                                        at[:, j * 512 + dj : (j + 1) * 512],
                                        sp[:, j * 512 + dj : (j + 1) * 512],
                                        mybir.ActivationFunctionType.Exp,
                                        scale=0.125,
                                    )
                            if b >= 2 * qc:
                                # both tiles are diagonal: triangle blocks at
                                # cols d_lo and d_lo+640 (one strided mul)
                                mav = strided(at[:], d_lo, [[640, 2], [1, P]])
                                nc.vector.tensor_mul(mav, mav, trimask[:])
                            ats[b] = at
                            if b >= 1:
                                emit_pv(b - 1)
                            pop_filler()
                        emit_pv(nb - 1)
                        for sb in range(4):
                            t = qc * 4 + sb
                            rec = sm_pool.tile([P, 1], F32, tag="rec", name="rec")
                            nc.vector.reciprocal(rec[:], pv[sb][:, HD : HD + 1])
                            nc.vector.tensor_scalar_mul(
                                y[t][:, h * HD : (h + 1) * HD],
                                pv[sb][:, 0:HD],
                                rec[:],
                            )

                    with ExitStack() as xctx:
                        xt_pool = xctx.enter_context(tc.tile_pool(name="xt", bufs=1))
                        xT = [xt_pool.tile([P, T], IO_DT, tag=f"xT{k}", name=f"xT{k}") for k in range(KC)]
                        stage = xctx.enter_context(tc.tile_pool(name="stage", bufs=2))

                        # DMA queue order: v weights, first x chunk, group-0
                        # qk weights, rest of x — so the first v matmuls and
                        # the first attention group start as early as possible
                        wrv = [
                            stage.tile([P, KC, 256], IO_DT, tag=f"wrv{cc}", name=f"wrv{cc}")
                            for cc in range(2)
                        ]
                        for cc in range(2):
                            nc.sync.dma_start(
                                wrv[cc][:],
                                w3_in[
                                    :, 2 * HCOLS + cc * 256 : 2 * HCOLS + (cc + 1) * 256
                                ].rearrange("(kc p) m -> p kc m", p=P),
                            )

                        def emit_x_dma(tck):
                            for k in range(KC):
                                nc.sync.dma_start(
                                    xT[k][:, tck * 512 : (tck + 1) * 512],
                                    xt_in[k * P : (k + 1) * P, tck * 512 : (tck + 1) * 512],
                                )

                        def emit_wr_dma(m, part):
                            wr = stage.tile([P, KC, P], IO_DT, tag="wr", bufs=3, name="wr")
                            col0 = part * HCOLS + m * P
                            nc.sync.dma_start(
                                wr[:],
                                w3_in[:, col0 : col0 + P].rearrange(
                                    "(kc p) m -> p kc m", p=P
                                ),
                            )
                            return wr

                        emit_x_dma(0)
                        wr0 = [emit_wr_dma(0, 0), emit_wr_dma(0, 1)]
                        for tck in range(1, 4):
                            emit_x_dma(tck)

                        # unified psum layout for the whole body: one shared
                        # sp ring 2x[128,1024] (4 banks) carries v, qk, S and
                        # proj tiles; pv takes the other 4 banks
                        s_ps = qctx.enter_context(
                            tc.tile_pool(name="s_ps", bufs=2, space="PSUM")
                        )
                        pv_ps = qctx.enter_context(
                            tc.tile_pool(name="pv_ps", bufs=1, space="PSUM")
                        )

                        def v_tile(t):
                            nc.vector.memset(vaug[t][:, :, HD : HD + 1], 1.0)
                            for cc in range(2):
                                pt = s_ps.tile([P, 256], F32, tag="sp", name="pt")
                                for k in range(KC):
                                    nc.tensor.matmul(
                                        pt[:],
                                        xT[k][:, t * P : (t + 1) * P],
                                        wrv[cc][:, k, :],
                                        start=(k == 0),
                                        stop=(k == KC - 1),
                                    )
                                nc.vector.tensor_add(
                                    vaug[t][:, cc * 4 : (cc + 1) * 4, 0:HD],
                                    pt[:].rearrange("p (h d) -> p h d", d=HD),
                                    bvb[:, cc * 256 : (cc + 1) * 256].rearrange(
                                        "p (h d) -> p h d", d=HD
                                    ),
                                )

                        def qk_chunk(m, part, tck, wr):
                            dst = (qT, kT)[part]
                            bias = (bq_t, bk_t)[part]
                            pt = s_ps.tile([P, 512], F32, tag="sp", name="pt")
                            for k in range(KC):
                                nc.tensor.matmul(
                                    pt[:],
                                    wr[:, k, :],
                                    xT[k][:, tck * 512 : (tck + 1) * 512],
                                    start=(k == 0),
                                    stop=(k == KC - 1),
                                )
                            nc.vector.tensor_scalar_add(
                                dst[m][:, tck * 512 : (tck + 1) * 512],
                                pt[:],
                                bias[m][:],
                            )

                        def queue_qk(m):
                            for part in (0, 1):
                                wr = emit_wr_dma(m, part)
                                fillq.extend(
                                    lambda mm=m, pp=part, tt=tck, ww=wr: qk_chunk(
                                        mm, pp, tt, ww
                                    )
                                    for tck in range(4)
                                )

                        # front wavefront: v, group-0 qk and group-0
                        # attention advance together per token chunk, so ACT
                        # gets exp work ~30us earlier than a phased schedule
                        for tck in range(4):
                            for t in range(4 * tck, 4 * tck + 4):
                                v_tile(t)
                            qk_chunk(0, 0, tck, wr0[0])
                            qk_chunk(0, 1, tck, wr0[1])
                            if tck == 2:
                                queue_qk(1)
                            if upto >= 2:
                                attention(0, 0, tck)
                                attention(0, 1, tck)

                        for m in (1, 2):
                            drain_fillers()
                            queue_qk(m + 1)
                            if upto >= 2:
                                for qc in range(4):
                                    for h in (2 * m, 2 * m + 1):
                                        attention(m, h, qc)
                        drain_fillers()

                    if upto < 2:
                        return
                    # xT and staging freed; bring in proj buffers
                    yT = qctx.enter_context(tc.tile_pool(name="yt", bufs=1)).tile(
                        [P, 4, T], IO_DT, name="yT"
                    )
                    wpr = qctx.enter_context(tc.tile_pool(name="wpr", bufs=1)).tile(
                        [P, 4, C], IO_DT, name="wpr"
                    )
                    nc.sync.dma_start(
                        wpr[:], wp_in[:].rearrange("(kc p) m -> p kc m", p=P)
                    )
                    ob_pool = qctx.enter_context(tc.tile_pool(name="ob", bufs=3))

                    def proj_group(qc):
                        # transpose y tiles of this group, then proj + bias,
                        # then one pairwise reduce-scatter for the group
                        for sb in range(4):
                            t = qc * 4 + sb
                            tp = s_ps.tile([P, 512], IO_DT, tag="sp", name="tp")
                            for m in range(4):
                                nc.tensor.transpose(
                                    tp[:, m * P : (m + 1) * P],
                                    y[t][:, m * P : (m + 1) * P],
                                    ident[:],
                                )
                            # scatter the four column-chunks into yT[:, m, t]
                            nc.vector.tensor_copy(
                                strided(yT[:], t * P, [[T, 4], [1, P]]),
                                tp[:].rearrange("p (m c) -> p m c", c=P),
                            )
                        for sb in range(4):
                            t = qc * 4 + sb
                            pt = s_ps.tile([P, 1024], F32, tag="sp", name="pt")
                            for ncol in range(2):
                                for k in range(4):
                                    nc.tensor.matmul(
                                        pt[:, ncol * 512 : (ncol + 1) * 512],
                                        yT[:, k, t * P : (t + 1) * P],
                                        wpr[:, k, ncol * 512 : (ncol + 1) * 512],
                                        start=(k == 0),
                                        stop=(k == 3),
                                    )
                            ob = ob_pool.tile([P, C], rs_dt, tag="ob", name="ob")
                            nc.vector.tensor_add(ob[:], pt[:], bpb[:])
                            nc.sync.dma_start(
                                partial[qc][sb * P : (sb + 1) * P, :], ob[:]
                            )
                        if not no_rs:
                            nc.gpsimd.collective_compute(
                                "ReduceScatter",
                                mybir.AluOpType.add,
                                replica_groups=[[0, 1], [2, 3], [4, 5], [6, 7]],
                                ins=[partial[qc].opt()],
                                outs=[rs_out[qc].opt()],
                            )
                        nc.sync.dma_start(
                            out_p[qc * 256 : (qc + 1) * 256, :], rs_out[qc][:]
                        )

                    # group 3 attention with inline proj + reduce-scatter
                    for qc in range(4):
                        for h in (6, 7):
                            attention(3, h, qc)
                        if upto >= 3:
                            proj_group(qc)

        for _ in range(reps):
            body()

    legalize_waits(nc)
    return nc


def prep_inputs(x, W_qkv, b_qkv, W_proj, b_proj):
    x = np.asarray(x, dtype=np.float32)
    W_qkv = np.asarray(W_qkv, dtype=np.float32)
    b_qkv = np.asarray(b_qkv, dtype=np.float32)
    W_proj = np.asarray(W_proj, dtype=np.float32)
    b_proj = np.asarray(b_proj, dtype=np.float32)

    io_np = ml_dtypes.bfloat16 if IO_BF16 else np.float32
    xTs = [np.ascontiguousarray(x[b].T.astype(io_np)) for b in range(B)]
    halves = []
    for half in range(2):
        c0 = half * HCOLS
        w3 = np.ascontiguousarray(
            np.concatenate(
                [
                    W_qkv[:, c0 : c0 + HCOLS],
                    W_qkv[:, C + c0 : C + c0 + HCOLS],
                    W_qkv[:, 2 * C + c0 : 2 * C + c0 + HCOLS],
                ],
                axis=1,
            ).astype(io_np)
        )
        wp = np.ascontiguousarray(W_proj[c0 : c0 + HCOLS, :].astype(io_np))
        bq = np.ascontiguousarray(b_qkv[c0 : c0 + HCOLS].reshape(4, P, 1))
        bk = np.ascontiguousarray(b_qkv[C + c0 : C + c0 + HCOLS].reshape(4, P, 1))
        bvb = np.tile(b_qkv[2 * C + c0 : 2 * C + c0 + HCOLS], (P, 1))
        halves.append((w3, wp, bq, bk, np.ascontiguousarray(bvb)))
    # both cores of a pair add the proj bias before the ReduceScatter
    # sums them, so each adds half
    bpb = np.ascontiguousarray(np.tile(b_proj / 2.0, (P, 1)))

    kk = np.arange(P)[:, None]
    qq = np.arange(P)[None, :]
    masks = (kk <= qq).astype(ml_dtypes.bfloat16)[None]

    in_maps = []
    for c in range(NCORES):
        b, half = c // 2, c % 2
        w3, wp, bq, bk, bvb = halves[half]
        in_maps.append(
            {
                "xt": xTs[b],
                "w3": w3,
                "wp": wp,
                "bq": bq,
                "bk": bk,
                "bvb": bvb,
                "bpb": bpb,
                "masks": masks,
            }
        )
    return in_maps


class _Runner:
    """Build-once SPMD executor via PJRT (mirrors bass2jax.run_bass_via_pjrt)."""

    def __init__(self, nc, n_cores=NCORES):
        import jax
        from jax.sharding import Mesh, PartitionSpec, NamedSharding
        from jax.experimental.shard_map import shard_map
        from concourse.bass2jax import (
            _bass_exec_p,
            install_neuronx_cc_hook,
            partition_id_tensor,
        )

        self.jax = jax
        install_neuronx_cc_hook()
        partition_name = (
            nc.partition_id_tensor.name if nc.partition_id_tensor else None
        )
        in_names, out_names, out_avals, zero_outs = [], [], [], []
        for alloc in nc.m.functions[0].allocations:
            if not isinstance(alloc, mybir.MemoryLocationSet):
                continue
            name = alloc.memorylocations[0].name
            if alloc.kind == "ExternalInput":
                if name != partition_name:
                    in_names.append(name)
            elif alloc.kind == "ExternalOutput":
                shape = tuple(alloc.tensor_shape)
                dtype = mybir.dt.np(alloc.dtype)
                out_names.append(name)
                out_avals.append(jax.core.ShapedArray(shape, dtype))
                zero_outs.append(np.zeros(shape, dtype))
        self.in_names, self.out_names = in_names, out_names
        self.out_avals, self.zero_outs = out_avals, zero_outs
        self.n_cores = n_cores
        n_params = len(in_names)
        self.n_params = n_params
        all_in = list(in_names) + list(out_names)
        if partition_name is not None:
            all_in.append(partition_name)
        donate = tuple(range(n_params, n_params + len(out_names)))

        def _body(*args):
            operands = list(args)
            if partition_name is not None:
                operands.append(partition_id_tensor())
            outs = _bass_exec_p.bind(
                *operands,
                out_avals=tuple(out_avals),
                in_names=tuple(all_in),
                out_names=tuple(out_names),
                lowering_input_output_aliases=(),
                sim_require_finite=True,
                sim_require_nnan=True,
                nc=nc,
            )
            return tuple(outs)

        devices = jax.devices()[:n_cores]
        self.mesh = Mesh(np.asarray(devices), ("core",))
        in_specs = (PartitionSpec("core"),) * (n_params + len(out_names))
        out_specs = (PartitionSpec("core"),) * len(out_names)
        self.sharding = NamedSharding(self.mesh, PartitionSpec("core"))
        self.jitted = jax.jit(
            shard_map(
                _body,
                mesh=self.mesh,
                in_specs=in_specs,
                out_specs=out_specs,
                check_rep=False,
            ),
            donate_argnums=donate,
            keep_unused=True,
        )

    def put_inputs(self, in_maps):
        per_core = [[np.asarray(m[n]) for n in self.in_names] for m in in_maps]
        concat = [
            np.concatenate([per_core[c][i] for c in range(self.n_cores)], axis=0)
            for i in range(self.n_params)
        ]
        return [self.jax.device_put(a, self.sharding) for a in concat]

    def _zeros(self):
        return [
            self.jax.device_put(
                np.zeros((self.n_cores * z.shape[0], *z.shape[1:]), z.dtype),
                self.sharding,
            )
            for z in self.zero_outs
        ]

    def run(self, dev_inputs, n_timed=0):
        out = self.jitted(*dev_inputs, *self._zeros())
        self.jax.block_until_ready(out)
        times = []
        for _ in range(n_timed):
            z = self._zeros()
            self.jax.block_until_ready(z)
            t0 = time.perf_counter()
            out2 = self.jitted(*dev_inputs, *z)
            self.jax.block_until_ready(out2)
            times.append(time.perf_counter() - t0)
            out = out2
        np_outs = [np.asarray(a) for a in out]
        results = [
            {
                n: np_outs[i].reshape(self.n_cores, *self.out_avals[i].shape)[c]
                for i, n in enumerate(self.out_names)
            }
            for c in range(self.n_cores)
        ]
        return results, times


_RUNNERS = {}


def get_runner(reps: int = 1, **kw) -> _Runner:
    key = (reps, tuple(sorted(kw.items())))
    if key not in _RUNNERS:
        _RUNNERS[key] = _Runner(build_nc(reps, **kw))
    return _RUNNERS[key]


def kernel(x, W_qkv, b_qkv, W_proj, b_proj):
    in_maps = prep_inputs(x, W_qkv, b_qkv, W_proj, b_proj)
    runner = get_runner(1)
    results, _ = runner.run(runner.put_inputs(in_maps))
    out = np.empty((B, T, C), dtype=np.float32)
    for c in range(NCORES):
        b, rank = c // 2, c % 2
        part = results[c]["out_part"]
        for g in range(4):
            r0 = g * 512 + rank * 256
            out[b, r0 : r0 + 256, :] = np.asarray(
                part[g * 256 : (g + 1) * 256, :], dtype=np.float32
            )
    return out

